# revision 17
# baseline (speedup 1.0000x reference)
"""Trainium2 Bass kernel: masked contrastive loss, SPMD over 8 NeuronCores.

Math (reference: CustomContrastiveLoss):
  q = l2norm(logits.reshape(N,D)); k = l2norm(labels.reshape(N,D))
  sim = q @ k.T / TAU;  valid = pad_mask;  pos = (ad_i == ad_j) & valid_i & valid_j
  loss = mean_{valid rows} [ lse_valid(sim_row) - lse_pos(sim_row) ]
  (has_pos == valid because the diagonal is always a positive for valid rows)

Strategy:
  * Host sorts the valid samples by ad value (pure index manipulation).
    Invalid rows/cols drop out entirely and each row's positives become one
    contiguous column range.
  * Host also normalizes, folds 1/TAU into the query rows, casts to bf16 and
    lays out the PE-transposed operands directly ([K-dim partition, col free]),
    so the device runs zero pre-processing: no on-device normalization, casts,
    or PE transposes.  DMA descriptors are per-partition contiguous 2KB runs.
  * |sim| <= 1/TAU, so exp(sim - 1/TAU) needs no per-row max -> single pass:
    loss_row = ln(S_all) - ln(S_pos) with S = sum exp(sim - 1/TAU).
  * Rows are sharded across 8 cores. Every core gets all valid labels, but
    with its column order rotated by (core_row_offset - W) so each row
    tile's positives land in the fixed window [128t, 128t + Wtot) -- the
    SPMD program is identical across cores, variation is data-only.
  * Per core: PE matmul (bf16, full rate) -> ScalarE fused exp+row-sum from
    PSUM (bf16 est out) -> small DVE band ops for S_pos -> ln/sub/mask ->
    partition-sum.  Host adds the 8 partial sums and divides by the count.
"""

import math
import os
import sys

for _p in ("/opt/trn_rl_repo", "/root/.axon_site/_ro/trn_rl_repo"):
    if os.path.isdir(_p) and _p not in sys.path:
        sys.path.append(_p)

import numpy as np
import ml_dtypes

import concourse.bass as bass
import concourse.mybir as mybir
import concourse.tile as tile
from concourse.bass_utils import run_bass_kernel_spmd

TAU = 0.05
INV_TAU = 1.0 / TAU
EPS = 1e-12
P = 128
D = 256
KC = D // P
NCORES = 8
CHUNK = 512            # matmul moving width (one PSUM bank of fp32)
GR = 1024              # exp/accum group = one DMA chunk = 2 PSUM banks
DC = 1024              # DMA chunk columns (4KB/partition descriptors)
DCL = 512              # trimmed width of the last (ragged) chunk
F32 = mybir.dt.float32
BF16 = mybir.dt.bfloat16
NPBF = ml_dtypes.bfloat16
AF = mybir.ActivationFunctionType
OP = mybir.AluOpType

# ---------------------------------------------------------------------------
# This walrus build rejects more than one sync-wait per instruction.  After
# Tile scheduling, hoist excess waits onto same-engine NOPs inserted right
# before the over-subscribed instruction (engine streams are sequential, so
# the waits still happen-before the instruction).
_MAXW = 1
_wsplit_n = [0]


def _split_excess_waits(nc):
    for f in nc.m.functions:
        for bb in f.blocks:
            insts = bb.instructions
            i = 0
            while i < len(insts):
                inst = insts[i]
                si = getattr(inst, "sync_info", None)
                if si is not None and si.on_wait and len(si.on_wait) > _MAXW:
                    waits = list(si.on_wait)
                    si.on_wait = waits[:_MAXW]
                    rest = waits[_MAXW:]
                    for j in range(0, len(rest), _MAXW):
                        _wsplit_n[0] += 1
                        nop = mybir.InstNoOp(
                            name=f"wsplit-{_wsplit_n[0]}", ins=[], outs=[]
                        )
                        nop.engine = inst.engine
                        nop.sync_info = mybir.SyncInfo(
                            on_wait=rest[j : j + _MAXW], on_update=[]
                        )
                        insts.insert(i, nop)
                        i += 1
                i += 1


def _bcast_cols(ap, parts, lo, hi):
    """[1, C] DRAM AP -> [parts, hi-lo] AP with partition stride 0."""
    sl = ap[0:1, lo:hi]
    return bass.AP(tensor=sl.tensor, offset=sl.offset, ap=[[0, parts], sl.ap[-1]])


def build_program(V, Vp, R, Wtot, win_starts, cws):
    T = R // P
    ngr = (V + GR - 1) // GR
    nchunk = len(cws)
    ysoff = [KC * sum(cws[:i]) for i in range(nchunk + 1)]
    nc = bass.Bass("TRN2", target_bir_lowering=False, debug=False)
    qTd = nc.dram_tensor("qT", [P, KC * R], BF16, kind="ExternalInput")
    # label chunks interleaved [KC, cw] per partition: one contiguous
    # descriptor per (partition, chunk)
    ysd = nc.dram_tensor("ysT", [P, KC * Vp], BF16, kind="ExternalInput")
    adr = nc.dram_tensor("adr", [P, T], F32, kind="ExternalInput")
    mskd = nc.dram_tensor("masks", [P, T * Wtot], BF16, kind="ExternalInput")
    rmask = nc.dram_tensor("rmask", [P, T], F32, kind="ExternalInput")
    rpad = nc.dram_tensor("rpad", [P, T], F32, kind="ExternalInput")
    outp = nc.dram_tensor("partial", [1, 1], F32, kind="ExternalOutput")

    with tile.TileContext(nc) as tc:
        with (
            tc.tile_pool(name="singles", bufs=1) as singles,
            tc.tile_pool(name="tiny", bufs=6) as tiny,
            tc.tile_pool(name="est", bufs=2) as est_pool,
            tc.tile_pool(name="band", bufs=2) as band,
            tc.tile_pool(name="pmm", bufs=4, space="PSUM") as pmm,
        ):
            ones = singles.tile([P, 1], F32)
            nc.vector.memset(ones[:], 1.0)
            ones_bf = singles.tile([P, 1], BF16)
            nc.vector.memset(ones_bf[:], 1.0)
            warm = singles.tile([P, CHUNK], BF16)
            nc.vector.memset(warm[:], 0.0)
            b_shift = singles.tile([P, 1], F32)
            nc.vector.memset(b_shift[:], -INV_TAU)

            qT = singles.tile([P, KC * R], BF16)
            ysT = singles.tile([P, KC * Vp], BF16)
            adr_s = singles.tile([P, T], F32)
            rm_s = singles.tile([P, T], F32)
            rp_s = singles.tile([P, T], F32)
            sall = singles.tile([P, T], F32)
            spos = singles.tile([P, T], F32)
            masks = singles.tile([P, T * Wtot], BF16)

            # scalar: nothing before the exp/ln table preload, so the table
            # is resident as soon as the first sim group lands
            tbl = tiny.tile([P, 1], F32)
            nc.scalar.activation(out=tbl[:], in_=ones[:], func=AF.Exp,
                                 bias=0.0, scale=1.0)

            # big operands: queries first, then label chunks in consumption
            # order; tiny metadata rides behind gpsimd's chunks
            nc.sync.dma_start(out=qT[:], in_=qTd.ap())
            big = (nc.gpsimd, nc.sync)
            for cc in range(nchunk):
                big[cc % 2].dma_start(
                    out=ysT[:, ysoff[cc]:ysoff[cc + 1]],
                    in_=ysd.ap()[:, ysoff[cc]:ysoff[cc + 1]])
            nc.gpsimd.dma_start(out=adr_s[:], in_=adr.ap())
            nc.gpsimd.dma_start(out=rm_s[:], in_=rmask.ap())
            nc.gpsimd.dma_start(out=rp_s[:], in_=rpad.ap())
            nc.gpsimd.dma_start(out=masks[:], in_=mskd.ap())

            # PE p-state warmup while the first label chunks land
            for _ in range(3):
                pw = pmm.tile([P, GR], F32, tag="mm")
                nc.tensor.matmul(pw[0:1, 0:CHUNK], ones_bf[:], warm[:],
                                 start=True, stop=True)

            # ---- main loop: sim tile -> fused exp + row-sum -> band S_pos
            for t in range(T):
                est = est_pool.tile([P, Vp], BF16)
                sparts = tiny.tile([P, ngr], F32)
                for g in range(ngr):
                    c0 = g * GR
                    w = min(GR, V - c0)
                    ps = pmm.tile([P, GR], F32, tag="mm")
                    for off in range(0, w, CHUNK):
                        hw = min(CHUNK, w - off)
                        for kc in range(KC):
                            nc.tensor.matmul(
                                ps[:, off:off + hw],
                                qT[:, kc * R + t * P:kc * R + (t + 1) * P],
                                ysT[:, ysoff[g] + kc * cws[g] + off:
                                    ysoff[g] + kc * cws[g] + off + hw],
                                start=(kc == 0), stop=(kc == KC - 1),
                            )
                    # e = exp(dot - 20), row-sum into sparts[:, g]
                    nc.scalar.activation(out=est[:, c0:c0 + w], in_=ps[:, :w],
                                         func=AF.Exp, bias=b_shift[:], scale=1.0,
                                         accum_out=sparts[:, g:g + 1])
                nc.vector.tensor_reduce(out=sall[:, t:t + 1], in_=sparts[:],
                                        axis=mybir.AxisListType.X, op=OP.add)

                w0 = win_starts[t]
                scr = band.tile([P, Wtot], BF16)
                nc.vector.tensor_mul(out=scr[:], in0=est[:, w0:w0 + Wtot],
                                     in1=masks[:, t * Wtot:(t + 1) * Wtot])
                nc.vector.tensor_reduce(out=spos[:, t:t + 1], in_=scr[:],
                                        axis=mybir.AxisListType.X, op=OP.add)

            # ---- batched epilogue: loss rows, mask, partition sum
            sposg = tiny.tile([P, T], F32)
            nc.vector.tensor_add(out=sposg[:], in0=spos[:], in1=rp_s[:])
            lall = tiny.tile([P, T], F32)
            nc.scalar.activation(out=lall[:], in_=sall[:], func=AF.Ln,
                                 bias=0.0, scale=1.0)
            lpos = tiny.tile([P, T], F32)
            nc.scalar.activation(out=lpos[:], in_=sposg[:], func=AF.Ln,
                                 bias=0.0, scale=1.0)
            dls = tiny.tile([P, T], F32)
            nc.vector.tensor_sub(out=dls[:], in0=lall[:], in1=lpos[:])
            dlm = tiny.tile([P, T], F32)
            nc.vector.tensor_mul(out=dlm[:], in0=dls[:], in1=rm_s[:])
            accv = tiny.tile([P, 1], F32)
            nc.vector.tensor_reduce(out=accv[:], in_=dlm[:],
                                    axis=mybir.AxisListType.X, op=OP.add)
            pfin = pmm.tile([P, GR], F32, tag="mm")
            nc.tensor.matmul(pfin[0:1, 0:1], ones[:], accv[:],
                             start=True, stop=True)
            ot = tiny.tile([1, 1], F32)
            nc.vector.tensor_copy(out=ot[:], in_=pfin[0:1, 0:1])
            nc.sync.dma_start(out=outp.ap(), in_=ot[:])

    return nc


def _roundup(a, b):
    return (a + b - 1) // b * b


def plan(valid, ad):
    """Host-side sharding plan from the pad mask / ad ids (index math only)."""
    idx = np.nonzero(valid)[0]
    V = int(idx.size)
    if V == 0:
        return None
    order = idx[np.argsort(ad[idx], kind="stable")]
    ads = ad[order].astype(np.int64)
    R = _roundup(_roundup(V, NCORES) // NCORES, P)
    W = int(np.bincount(ads).max())
    Wtot = min(_roundup(2 * W + P, 32), V)
    T = R // P
    rotate = (R - P + Wtot <= V) and Wtot < V
    if rotate:
        win_starts = tuple(min(t * P, V - Wtot) for t in range(T))
    else:
        Wtot = V
        win_starts = (0,) * T
    cws = [DC] * (V // DC)
    rem = V % DC
    if rem:
        cws.append(DCL if rem <= DCL else DC)
    Vp = sum(cws)
    return dict(V=V, Vp=Vp, R=R, T=T, W=W, Wtot=Wtot, win_starts=win_starts,
                rotate=rotate, order=order, ads=ads, cws=tuple(cws))


def host_prep(pl, x, y):
    """Normalize, fold 1/TAU into queries, cast bf16, build transposed
    layouts shared across cores (host indexing + elementwise prep only)."""
    order = pl["order"]
    V = pl["V"]
    xn = x[order]
    xnrm = np.sqrt(np.sum(xn * xn, axis=1, keepdims=True))
    qsc = (xn * (INV_TAU / np.maximum(xnrm, EPS))).astype(NPBF)   # [V, D]
    yn = y[order]
    ynrm = np.sqrt(np.sum(yn * yn, axis=1, keepdims=True))
    ksc = (yn / np.maximum(ynrm, EPS)).astype(NPBF)               # [V, D]
    # sorted transposed labels: ysT0[p, kc, v] = ksc[v, kc*P + p]
    ysT0 = np.ascontiguousarray(ksc.T.reshape(KC, P, V).transpose(1, 0, 2))
    return qsc, ysT0


def core_inputs(pl, qsc, ysT0, c):
    """Build core c's input arrays from the plan (host indexing only)."""
    V, Vp, R, W, T = pl["V"], pl["Vp"], pl["R"], pl["W"], pl["T"]
    ads = pl["ads"]
    g0 = c * R
    nv = max(0, min(R, V - g0))

    # queries: qT[p, kc*R + r] = qsc[g0 + r, kc*P + p]
    qT = np.zeros((P, KC, R), NPBF)
    if nv > 0:
        qT[:, :, :nv] = qsc[g0:g0 + nv].T.reshape(KC, P, nv).transpose(1, 0, 2)
    qT = np.ascontiguousarray(qT.reshape(P, KC * R))

    # labels: rotate sorted columns by (g0 - W) so each row tile's positives
    # land in its fixed window
    if pl["rotate"]:
        shift = (g0 - W) % V
        ysc = np.roll(ysT0, -shift, axis=2)
        adc_c = np.roll(ads, -shift)
    else:
        ysc = ysT0
        adc_c = ads
    # interleaved chunk layout: per chunk cc, [KC, cw] contiguous/partition
    cws = pl["cws"]
    ysF = np.zeros((P, KC, Vp), NPBF)
    ysF[:, :, :V] = ysc
    parts, off = [], 0
    for cw in cws:
        parts.append(np.ascontiguousarray(
            ysF[:, :, off:off + cw]).reshape(P, KC * cw))
        off += cw
    ysT = np.ascontiguousarray(np.concatenate(parts, axis=1))

    adr_flat = np.full(R, -1.0, np.float32)
    adr_flat[:nv] = ads[g0:g0 + nv]
    rmask_flat = np.zeros(R, np.float32)
    rmask_flat[:nv] = 1.0
    # packed [P, T]: column t holds rows [t*P, (t+1)*P) of this core's shard
    adr = np.ascontiguousarray(adr_flat.reshape(T, P).T)
    rmask = np.ascontiguousarray(rmask_flat.reshape(T, P).T)
    rpad = np.ascontiguousarray(1.0 - rmask)

    # band masks on host: masks[p, t*Wtot + j] = (adc[w0_t + j] == adr[p, t])
    adc_pad = np.full(Vp, -2.0, np.float64)
    adc_pad[:V] = adc_c
    Wtot = pl["Wtot"]
    masks = np.zeros((P, T, Wtot), NPBF)
    for t in range(T):
        w0 = pl["win_starts"][t]
        masks[:, t, :] = (adc_pad[None, w0:w0 + Wtot] == adr[:, t:t + 1])
    masks = np.ascontiguousarray(masks.reshape(P, T * Wtot))
    return {"qT": qT, "ysT": ysT, "adr": adr, "masks": masks,
            "rmask": rmask, "rpad": rpad}


_prog_cache = {}


def _get_program(pl):
    key = (pl["V"], pl["Vp"], pl["R"], pl["Wtot"], pl["win_starts"])
    if key not in _prog_cache:
        _prog_cache[key] = build_program(
            pl["V"], pl["Vp"], pl["R"], pl["Wtot"], pl["win_starts"],
            pl["cws"]
        )
    return _prog_cache[key]


def kernel(logits, labels, pad_mask, ad_idxs, _want_results=False, **run_kwargs):
    x = np.ascontiguousarray(np.asarray(logits), dtype=np.float32).reshape(-1, D)
    y = np.ascontiguousarray(np.asarray(labels), dtype=np.float32).reshape(-1, D)
    valid = np.asarray(pad_mask).reshape(-1).astype(bool)
    ad = np.asarray(ad_idxs).reshape(-1).astype(np.int64)

    pl = plan(valid, ad)
    if pl is None:
        return np.float32(0.0)

    nc = _get_program(pl)
    # CoreSim chokes on the inserted NOPs, so split waits only for the HW path
    if not getattr(nc, "_waits_split", False):
        _split_excess_waits(nc)
        nc._waits_split = True
    qsc, ysT0 = host_prep(pl, x, y)
    in_maps = [core_inputs(pl, qsc, ysT0, c) for c in range(NCORES)]
    res = run_bass_kernel_spmd(nc, in_maps, core_ids=list(range(NCORES)),
                               **run_kwargs)
    total = sum(float(res.results[c]["partial"][0, 0]) for c in range(NCORES))
    loss = np.float32(total / pl["V"])
    if _want_results:
        return loss, res
    return loss
